# revision 11
# baseline (speedup 1.0000x reference)
"""Bahdanau attention (audio) kernel for 8x Trainium2 NeuronCores.

Strategy (per sharding hint): data-parallel over batch B=32 across 8 cores
(4 batches/core); all params replicated. Per batch, on-device:

  sum_1^T[u, t] = sum_h W1[u,h] * v[t,h]   (bf16 PE matmul, fp32 PSUM accum)
                + sum_j G[u,j] * P[j,t]    (G = loc_proj @ conv_w folded once,
                                            P = shifted prev_att windows)
  tanh via ACT with per-partition bias = (q @ W2^T + W1_b + W2_b)[u]
  score^T[t]  = sum_u tanh[u,t] * V_w[u]   (PE matvec accum)
  softmax over t (free dim) on [1, T] row; context = sum_t w[t]*v[t,h] via
  DVE tensor_tensor_reduce over transposed values.

values^T is materialized via DRAM f32->bf16 cast DMA (SWDGE) + one xbar
DMA-transpose per batch ([T,H] bf16 -> SBUF [128, H/128, T], h = c*128+p).
"""

from contextlib import ExitStack

import numpy as np

import concourse.bass as bass
import concourse.tile as tile
import concourse.mybir as mybir
from concourse import bacc
from concourse.bass import AP
from concourse.bass_utils import run_bass_kernel_spmd
from concourse.masks import make_identity

B, T, H, U = 32, 2048, 1024, 1024
KN, KW, PAD = 32, 31, 16
NCORES = 8
BL = B // NCORES
HC, UC = H // 128, U // 128
THS = 1024  # T-half tile size for PSUM
F32, BF16 = mybir.dt.float32, mybir.dt.bfloat16
AX = mybir.AxisListType.X
AF = mybir.ActivationFunctionType
OP = mybir.AluOpType


def build_kernel(ctx, nc, tc, io, bl=BL):
    values, query, prev = io["values"], io["query"], io["prev"]
    W1_w, W2_w, W1_b, W2_b = io["W1_w"], io["W2_w"], io["W1_b"], io["W2_b"]
    V_w, V_b, conv_w, loc_proj = io["V_w"], io["V_b"], io["conv_w"], io["loc_proj"]
    ctx_out, attn_out, score_out = io["ctx_out"], io["attn_out"], io["score_out"]

    TPADDED = T + 2 * PAD

    # ---------------- whole-kernel pools ----------------
    const = ctx.enter_context(tc.tile_pool(name="const", bufs=1))
    dram = ctx.enter_context(tc.tile_pool(name="dram", bufs=1, space="DRAM"))
    vt_pool = ctx.enter_context(tc.tile_pool(name="vt", bufs=2))
    work = ctx.enter_context(tc.tile_pool(name="work", bufs=2))
    rows = ctx.enter_context(tc.tile_pool(name="rows", bufs=1))

    W1T = const.tile([128, HC, U], BF16)       # W1T[p, c, u] = W1[u, c*128+p]
    GT16 = const.tile([KW, U], BF16)           # GT[j, u] = G[u, j]
    VT16 = const.tile([128, UC], BF16)         # VT[p, uc] = V_w[uc*128+p]
    biasT = const.tile([128, UC, bl], F32)     # (qW2 + W1b + W2b)[u] per batch
    vb_sb = const.tile([1, 1], F32)
    prevpad = dram.tile([BL, TPADDED], F32)

    # ---------------- startup: weights prep ----------------
    with (
        tc.tile_pool(name="su_sbuf", bufs=1) as su,
        tc.tile_pool(name="su_psum", bufs=1, space="PSUM") as sups,
        tc.tile_pool(name="su_dram", bufs=1, space="DRAM") as sudram,
    ):
        # f32 -> bf16 DRAM copies of W1/W2, then xbar-transpose into SBUF
        w1_16 = sudram.tile([U, H], BF16)
        w2_16 = sudram.tile([U, H], BF16)
        nc.gpsimd.dma_start(out=w1_16[:], in_=W1_w[:])
        nc.gpsimd.dma_start(out=w2_16[:], in_=W2_w[:])
        W2T = su.tile([128, HC, U], BF16)
        nc.sync.dma_start_transpose(W1T[:], w1_16[:])
        nc.sync.dma_start_transpose(W2T[:], w2_16[:])

        # identity for PE transposes
        ident = su.tile([128, 128], F32)
        make_identity(nc, ident[:])

        # lpT[k, u] = loc_proj[u, k] via PE transposes of natural tiles
        lpT = su.tile([KN, U], F32)
        for uc in range(UC):
            lp_nat = su.tile([128, KN], F32, tag="lp_nat", bufs=2)
            nc.sync.dma_start(out=lp_nat[:], in_=loc_proj[uc * 128:(uc + 1) * 128, :])
            lp_ps = sups.tile([KN, 128], F32, tag="lp_ps", bufs=2)
            nc.tensor.transpose(lp_ps[:], lp_nat[:], ident[:])
            nc.vector.tensor_copy(lpT[:, uc * 128:(uc + 1) * 128], lp_ps[:])

        # GT = (loc_proj @ conv_w)^T = cw^T-contract: GT[j,u] = sum_k cw[k,j] lp[u,k]
        cw_sb = su.tile([KN, KW], F32)
        nc.sync.dma_start(out=cw_sb[:], in_=conv_w[:])
        gt_ps = sups.tile([KW, U], F32, tag="gt_ps", bufs=1)
        for s in range(U // 512):
            nc.tensor.matmul(gt_ps[:, s * 512:(s + 1) * 512], cw_sb[:],
                             lpT[:, s * 512:(s + 1) * 512], start=True, stop=True)
        nc.vector.tensor_copy(GT16[:], gt_ps[:])

        # V_w -> [128, UC] chunks, bf16
        v_f32 = su.tile([128, UC], F32)
        for uc in range(UC):
            nc.sync.dma_start(out=v_f32[:, uc:uc + 1],
                              in_=V_w[0, uc * 128:(uc + 1) * 128])
        nc.vector.tensor_copy(VT16[:], v_f32[:])
        nc.sync.dma_start(out=vb_sb[:], in_=V_b[:])

        # qT[p, c, b] = query[b, c*128+p]  (32 tiny contiguous DMAs)
        qT_f32 = su.tile([128, HC, bl], F32)
        for c in range(HC):
            for b in range(bl):
                nc.sync.dma_start(out=qT_f32[:, c, b:b + 1],
                                  in_=query[b, c * 128:(c + 1) * 128])
        qT16 = su.tile([128, HC, bl], BF16)
        nc.vector.tensor_copy(qT16[:], qT_f32[:])

        # per-u bias vectors: W1_b + W2_b chunks
        b1 = su.tile([128, UC], F32)
        b2 = su.tile([128, UC], F32)
        for uc in range(UC):
            nc.sync.dma_start(out=b1[:, uc:uc + 1], in_=W1_b[uc * 128:(uc + 1) * 128])
            nc.sync.dma_start(out=b2[:, uc:uc + 1], in_=W2_b[uc * 128:(uc + 1) * 128])
        bsum = su.tile([128, UC], F32)
        nc.vector.tensor_add(bsum[:], b1[:], b2[:])

        # qW2^T chunks + bias -> biasT
        for uc in range(UC):
            qw_ps = sups.tile([128, bl], F32, tag="qw_ps", bufs=2)
            for c in range(HC):
                nc.tensor.matmul(qw_ps[:], W2T[:, c, uc * 128:(uc + 1) * 128],
                                 qT16[:, c, :], start=(c == 0), stop=(c == HC - 1))
            nc.vector.tensor_add(biasT[:, uc, :], qw_ps[:],
                                 bsum[:, uc:uc + 1].broadcast_to([128, bl]))

        # prevpad: zero edges + payload rows (DRAM scratch)
        zrow = su.tile([1, PAD], F32)
        nc.vector.memset(zrow[:], 0.0)
        for b in range(bl):
            nc.sync.dma_start(out=prevpad[b, 0:PAD], in_=zrow[:])
            nc.sync.dma_start(out=prevpad[b, T + PAD:TPADDED], in_=zrow[:])
            nc.sync.dma_start(out=prevpad[b, PAD:PAD + T], in_=prev[b, :])

    # ---------------- main PSUM pools ----------------
    psmain = ctx.enter_context(tc.tile_pool(name="ps_main", bufs=2, space="PSUM"))
    pssc = ctx.enter_context(tc.tile_pool(name="ps_sc", bufs=2, space="PSUM"))

    pp_ap = prevpad[:]

    # ---------------- per-batch pipeline ----------------
    for b in range(bl):
        # stage inputs: bf16 cast of values to DRAM, then xbar transpose
        v16 = dram.tile([T, H], BF16, tag="v16", bufs=2)
        nc.gpsimd.dma_start(out=v16[:], in_=values[b])
        vT = vt_pool.tile([128, HC, T], BF16)        # vT[p,c,t] = v[t, c*128+p]
        nc.sync.dma_start_transpose(vT[:], v16[:])

        # P[j, t] = prev_padded[b, 1 + j + t], cast to bf16 during DMA
        P16 = work.tile([KW, T], BF16, tag="p16")
        src = AP(pp_ap.tensor, pp_ap.offset + b * TPADDED + 1, [[1, KW], [1, T]])
        nc.gpsimd.dma_start(out=P16[:], in_=src)

        score_sb = rows.tile([1, T], F32, tag="score")
        for th in range(T // THS):
            t0 = th * THS
            ps_sc = pssc.tile([1, THS], F32)
            for ut in range(UC):
                ps = psmain.tile([128, THS], F32)
                u0 = ut * 128
                # K-groups: conv/loc first (K=31), then 8 h-chunks (K=128)
                for ki in range(HC + 1):
                    for s2 in range(THS // 512):
                        o = s2 * 512
                        if ki == 0:
                            lhsT = GT16[:, u0:u0 + 128]
                            rhs = P16[:, t0 + o:t0 + o + 512]
                        else:
                            lhsT = W1T[:, ki - 1, u0:u0 + 128]
                            rhs = vT[:, ki - 1, t0 + o:t0 + o + 512]
                        nc.tensor.matmul(ps[:, o:o + 512], lhsT, rhs,
                                         start=(ki == 0), stop=(ki == HC))
                th16 = work.tile([128, THS], BF16, tag="tanh", bufs=3)
                nc.scalar.activation(th16[:], ps[:], AF.Tanh,
                                     bias=biasT[:, ut, b:b + 1], scale=1.0)
                for s2 in range(THS // 512):
                    o = s2 * 512
                    nc.tensor.matmul(ps_sc[:, o:o + 512], VT16[:, ut:ut + 1],
                                     th16[:, o:o + 512],
                                     start=(ut == 0), stop=(ut == UC - 1),
                                     skip_group_check=True)
            nc.vector.tensor_copy(score_sb[:, t0:t0 + THS], ps_sc[:])

        # softmax over T (free dim) on a [1, T] row
        m_neg = rows.tile([1, 1], F32, tag="mneg")
        nc.vector.reduce_max(m_neg[:], score_sb[:], axis=AX, negate=True)
        e_sb = rows.tile([1, T], F32, tag="esb")
        ssum = rows.tile([1, 1], F32, tag="ssum")
        nc.scalar.activation(e_sb[:], score_sb[:], AF.Exp, bias=m_neg[:],
                             scale=1.0, accum_out=ssum[:])
        rinv = rows.tile([1, 1], F32, tag="rinv")
        nc.vector.reciprocal(rinv[:], ssum[:])
        w_sb = rows.tile([1, T], F32, tag="wsb")
        nc.vector.tensor_scalar_mul(w_sb[:], e_sb[:], rinv[:])
        nc.sync.dma_start(out=attn_out[b, :], in_=w_sb[:])

        sco = rows.tile([1, T], F32, tag="sco")
        nc.vector.tensor_scalar_add(sco[:], score_sb[:], vb_sb[:])
        nc.sync.dma_start(out=score_out[b, :], in_=sco[:])

        # context: ctx[h=c*128+p] = sum_t w[t] * vT[p, c, t]
        w16 = rows.tile([1, T], BF16, tag="w16")
        nc.vector.tensor_copy(w16[:], w_sb[:])
        w_bc = work.tile([128, T], BF16, tag="wbc")
        nc.gpsimd.partition_broadcast(w_bc[:], w16[:])
        ctx_sb = work.tile([128, HC], F32, tag="ctx")
        for c in range(HC):
            tts = work.tile([128, T], BF16, tag="tts", bufs=2)
            nc.vector.tensor_mul(tts[:], vT[:, c, :], w_bc[:])
            nc.vector.reduce_sum(ctx_sb[:, c:c + 1], tts[:], axis=AX)
        for c in range(HC):
            nc.sync.dma_start(out=ctx_out[b, c * 128:(c + 1) * 128],
                              in_=ctx_sb[:, c:c + 1])


def build_program(bl=BL):
    nc = bacc.Bacc("TRN2", target_bir_lowering=False, debug=False,
                   enable_asserts=False)
    io = {}

    def inp(name, shape):
        io[name] = nc.dram_tensor(name, list(shape), F32, kind="ExternalInput").ap()

    def outp(name, shape):
        io[name] = nc.dram_tensor(name, list(shape), F32, kind="ExternalOutput").ap()

    inp("values", (bl, T, H))
    inp("query", (bl, H))
    inp("prev", (bl, T))
    inp("W1_w", (U, H))
    inp("W2_w", (U, H))
    inp("W1_b", (U,))
    inp("W2_b", (U,))
    inp("V_w", (1, U))
    inp("V_b", (1,))
    inp("conv_w", (KN, KW))
    inp("loc_proj", (U, KN))
    outp("ctx_out", (bl, H))
    outp("attn_out", (bl, T))
    outp("score_out", (bl, T))

    with tile.TileContext(nc) as tc, ExitStack() as ctx:
        build_kernel(ctx, nc, tc, io, bl=bl)
    nc.compile()
    return nc


def make_in_maps(query, values, prev_att, W1_w, W1_b, W2_w, W2_b, V_w, V_b,
                 conv_w, loc_proj_w, ncores=NCORES, bl=BL):
    f = lambda x: np.ascontiguousarray(np.asarray(x), dtype=np.float32)
    shared = {
        "W1_w": f(W1_w), "W2_w": f(W2_w), "W1_b": f(W1_b), "W2_b": f(W2_b),
        "V_w": f(V_w).reshape(1, U), "V_b": f(V_b).reshape(1),
        "conv_w": f(conv_w).reshape(KN, KW), "loc_proj": f(loc_proj_w),
    }
    in_maps = []
    for core in range(ncores):
        sl = slice(core * bl, (core + 1) * bl)
        m = dict(shared)
        m["values"] = f(values[sl])
        m["query"] = f(query[0, sl])
        m["prev"] = f(prev_att[sl, 0, :])
        in_maps.append(m)
    return in_maps


_PROGRAM = None


def _get_program():
    global _PROGRAM
    if _PROGRAM is None:
        _PROGRAM = build_program()
    return _PROGRAM


def kernel(query, values, prev_att, W1_w, W1_b, W2_w, W2_b, V_w, V_b,
           conv_w, loc_proj_w):
    nc = _get_program()
    in_maps = make_in_maps(query, values, prev_att, W1_w, W1_b, W2_w, W2_b,
                           V_w, V_b, conv_w, loc_proj_w)
    res = run_bass_kernel_spmd(nc, in_maps, list(range(NCORES)))
    ctx = np.concatenate([res.results[i]["ctx_out"] for i in range(NCORES)], 0)
    attn = np.concatenate([res.results[i]["attn_out"] for i in range(NCORES)], 0)
    score = np.concatenate([res.results[i]["score_out"] for i in range(NCORES)], 0)
    return (ctx.astype(np.float32), attn[:, :, None].astype(np.float32),
            score[:, :, None].astype(np.float32))


# revision 15
# speedup vs baseline: 7.0829x; 7.0829x over previous
"""Bahdanau attention (audio) kernel for 8x Trainium2 NeuronCores.

Strategy (per sharding hint): data-parallel over batch B=32 across 8 cores
(4 batches/core); all params replicated. Per batch, on-device:

  sum_1^T[u, t] = sum_h W1[u,h] * v[t,h]   (bf16 PE matmul, fp32 PSUM accum)
                + sum_j G[u,j] * P[j,t]    (G = loc_proj @ conv_w folded once,
                                            P = shifted prev_att windows)
  tanh via ACT with per-partition bias = (q @ W2^T + W1_b + W2_b)[u]
  score^T[t]  = sum_u tanh[u,t] * V_w[u]   (PE matvec accum)
  softmax over t (free dim) on [1, T] row; context = sum_t w[t]*v[t,h] via
  DVE tensor_tensor_reduce over transposed values.

values^T is materialized via DRAM f32->bf16 cast DMA (SWDGE) + one xbar
DMA-transpose per batch ([T,H] bf16 -> SBUF [128, H/128, T], h = c*128+p).
"""

from contextlib import ExitStack

import numpy as np

import concourse.bass as bass
import concourse.tile as tile
import concourse.mybir as mybir
from concourse import bacc
from concourse.bass import AP
from concourse.bass_utils import run_bass_kernel_spmd
from concourse.masks import make_identity

B, T, H, U = 32, 2048, 1024, 1024
KN, KW, PAD = 32, 31, 16
NCORES = 8
BL = B // NCORES
HC, UC = H // 128, U // 128
THS = 1024  # T-half tile size for PSUM
F32, BF16 = mybir.dt.float32, mybir.dt.bfloat16
AX = mybir.AxisListType.X
AF = mybir.ActivationFunctionType
OP = mybir.AluOpType


def build_kernel(ctx, nc, tc, io, bl=BL, reps=1):
    values, query, prev = io["values"], io["query"], io["prev"]
    W1_w, W2_w, W1_b, W2_b = io["W1_w"], io["W2_w"], io["W1_b"], io["W2_b"]
    V_w, V_b, conv_w, loc_proj = io["V_w"], io["V_b"], io["conv_w"], io["loc_proj"]
    ctx_out, attn_out, score_out = io["ctx_out"], io["attn_out"], io["score_out"]

    TPADDED = T + 2 * PAD

    # ---------------- whole-kernel pools ----------------
    const = ctx.enter_context(tc.tile_pool(name="const", bufs=1))
    dram = ctx.enter_context(tc.tile_pool(name="dram", bufs=1, space="DRAM"))
    vt_pool = ctx.enter_context(tc.tile_pool(name="vt", bufs=2))
    work = ctx.enter_context(tc.tile_pool(name="work", bufs=2))
    rows = ctx.enter_context(tc.tile_pool(name="rows", bufs=1))

    W1T = const.tile([128, HC, U], BF16)       # W1T[p, c, u] = W1[u, c*128+p]
    GT16 = const.tile([KW, U], BF16)           # GT[j, u] = G[u, j]
    VT16 = const.tile([128, UC], BF16)         # VT[p, uc] = V_w[uc*128+p]
    biasT = const.tile([128, UC, bl], F32)     # (qW2 + W1b + W2b)[u] per batch
    vb_sb = const.tile([1, 1], F32)
    prevpad = dram.tile([BL, TPADDED], F32)

    # ---------------- startup: weights prep ----------------
    with (
        tc.tile_pool(name="su_sbuf", bufs=1) as su,
        tc.tile_pool(name="su_psum", bufs=1, space="PSUM") as sups,
        tc.tile_pool(name="su_dram", bufs=1, space="DRAM") as sudram,
    ):
        # f32 -> bf16 DRAM copies of W1/W2, then xbar-transpose into SBUF
        w1_16 = sudram.tile([U, H], BF16)
        w2_16 = sudram.tile([U, H], BF16)
        nc.gpsimd.dma_start(out=w1_16[:], in_=W1_w[:])
        nc.gpsimd.dma_start(out=w2_16[:], in_=W2_w[:])
        W2T = su.tile([128, HC, U], BF16)
        nc.sync.dma_start_transpose(W1T[:], w1_16[:])
        nc.sync.dma_start_transpose(W2T[:], w2_16[:])

        # identity for PE transposes
        ident = su.tile([128, 128], F32)
        make_identity(nc, ident[:])

        # lpT[k, u] = loc_proj[u, k] via PE transposes of natural tiles
        lpT = su.tile([KN, U], F32)
        for uc in range(UC):
            lp_nat = su.tile([128, KN], F32, tag="lp_nat", bufs=2)
            nc.sync.dma_start(out=lp_nat[:], in_=loc_proj[uc * 128:(uc + 1) * 128, :])
            lp_ps = sups.tile([KN, 128], F32, tag="lp_ps", bufs=2)
            nc.tensor.transpose(lp_ps[:], lp_nat[:], ident[:])
            nc.vector.tensor_copy(lpT[:, uc * 128:(uc + 1) * 128], lp_ps[:])

        # GT = (loc_proj @ conv_w)^T = cw^T-contract: GT[j,u] = sum_k cw[k,j] lp[u,k]
        cw_sb = su.tile([KN, KW], F32)
        nc.sync.dma_start(out=cw_sb[:], in_=conv_w[:])
        gt_ps = sups.tile([KW, U], F32, tag="gt_ps", bufs=1)
        for s in range(U // 512):
            nc.tensor.matmul(gt_ps[:, s * 512:(s + 1) * 512], cw_sb[:],
                             lpT[:, s * 512:(s + 1) * 512], start=True, stop=True)
        nc.vector.tensor_copy(GT16[:], gt_ps[:])

        # V_w -> [128, UC] chunks, bf16
        v_f32 = su.tile([128, UC], F32)
        for uc in range(UC):
            nc.sync.dma_start(out=v_f32[:, uc:uc + 1],
                              in_=V_w[0, uc * 128:(uc + 1) * 128])
        nc.vector.tensor_copy(VT16[:], v_f32[:])
        nc.sync.dma_start(out=vb_sb[:], in_=V_b[:])

        # qT[p, c, b] = query[b, c*128+p]  (32 tiny contiguous DMAs)
        qT_f32 = su.tile([128, HC, bl], F32)
        for c in range(HC):
            for b in range(bl):
                nc.sync.dma_start(out=qT_f32[:, c, b:b + 1],
                                  in_=query[b, c * 128:(c + 1) * 128])
        qT16 = su.tile([128, HC, bl], BF16)
        nc.vector.tensor_copy(qT16[:], qT_f32[:])

        # per-u bias vectors: W1_b + W2_b chunks
        b1 = su.tile([128, UC], F32)
        b2 = su.tile([128, UC], F32)
        for uc in range(UC):
            nc.sync.dma_start(out=b1[:, uc:uc + 1], in_=W1_b[uc * 128:(uc + 1) * 128])
            nc.sync.dma_start(out=b2[:, uc:uc + 1], in_=W2_b[uc * 128:(uc + 1) * 128])
        bsum = su.tile([128, UC], F32)
        nc.vector.tensor_add(bsum[:], b1[:], b2[:])

        # qW2^T chunks + bias -> biasT
        for uc in range(UC):
            qw_ps = sups.tile([128, bl], F32, tag="qw_ps", bufs=2)
            for c in range(HC):
                nc.tensor.matmul(qw_ps[:], W2T[:, c, uc * 128:(uc + 1) * 128],
                                 qT16[:, c, :], start=(c == 0), stop=(c == HC - 1))
            nc.vector.tensor_add(biasT[:, uc, :], qw_ps[:],
                                 bsum[:, uc:uc + 1].broadcast_to([128, bl]))

        # prevpad: zero edges + payload rows (DRAM scratch)
        zrow = su.tile([1, PAD], F32)
        nc.vector.memset(zrow[:], 0.0)
        for b in range(bl):
            nc.sync.dma_start(out=prevpad[b, 0:PAD], in_=zrow[:])
            nc.sync.dma_start(out=prevpad[b, T + PAD:TPADDED], in_=zrow[:])
            nc.sync.dma_start(out=prevpad[b, PAD:PAD + T], in_=prev[b, :])

    # ---------------- main PSUM pools ----------------
    psmain = ctx.enter_context(tc.tile_pool(name="ps_main", bufs=2, space="PSUM"))
    pssc = ctx.enter_context(tc.tile_pool(name="ps_sc", bufs=2, space="PSUM"))

    pp_ap = prevpad[:]

    # ---------------- per-batch pipeline ----------------
    for b in [b for _ in range(reps) for b in range(bl)]:
        # stage inputs: bf16 cast of values to DRAM, then xbar transpose
        v16 = dram.tile([T, H], BF16, tag="v16", bufs=2)
        nc.gpsimd.dma_start(out=v16[:], in_=values[b])
        vT = vt_pool.tile([128, HC, T], BF16)        # vT[p,c,t] = v[t, c*128+p]
        nc.sync.dma_start_transpose(vT[:], v16[:])

        # P[j, t] = prev_padded[b, 1 + j + t], cast to bf16 during DMA
        P16 = work.tile([KW, T], BF16, tag="p16")
        src = AP(pp_ap.tensor, pp_ap.offset + b * TPADDED + 1, [[1, KW], [1, T]])
        nc.gpsimd.dma_start(out=P16[:], in_=src)

        score_sb = rows.tile([1, T], F32, tag="score")
        for th in range(T // THS):
            t0 = th * THS
            ps_sc = pssc.tile([1, THS], F32)
            for ut in range(UC):
                ps = psmain.tile([128, THS], F32)
                u0 = ut * 128
                # K-groups: conv/loc first (K=31), then 8 h-chunks (K=128)
                for ki in range(HC + 1):
                    for s2 in range(THS // 512):
                        o = s2 * 512
                        if ki == 0:
                            lhsT = GT16[:, u0:u0 + 128]
                            rhs = P16[:, t0 + o:t0 + o + 512]
                        else:
                            lhsT = W1T[:, ki - 1, u0:u0 + 128]
                            rhs = vT[:, ki - 1, t0 + o:t0 + o + 512]
                        nc.tensor.matmul(ps[:, o:o + 512], lhsT, rhs,
                                         start=(ki == 0), stop=(ki == HC))
                th16 = work.tile([128, THS], BF16, tag="tanh", bufs=3)
                nc.scalar.activation(th16[:], ps[:], AF.Tanh,
                                     bias=biasT[:, ut, b:b + 1], scale=1.0)
                for s2 in range(THS // 512):
                    o = s2 * 512
                    nc.tensor.matmul(ps_sc[:, o:o + 512], VT16[:, ut:ut + 1],
                                     th16[:, o:o + 512],
                                     start=(ut == 0), stop=(ut == UC - 1),
                                     skip_group_check=True)
            nc.vector.tensor_copy(score_sb[:, t0:t0 + THS], ps_sc[:])

        # softmax over T (free dim) on a [1, T] row
        m_neg = rows.tile([1, 1], F32, tag="mneg")
        nc.vector.reduce_max(m_neg[:], score_sb[:], axis=AX, negate=True)
        e_sb = rows.tile([1, T], F32, tag="esb")
        ssum = rows.tile([1, 1], F32, tag="ssum")
        nc.scalar.activation(e_sb[:], score_sb[:], AF.Exp, bias=m_neg[:],
                             scale=1.0, accum_out=ssum[:])
        rinv = rows.tile([1, 1], F32, tag="rinv")
        nc.vector.reciprocal(rinv[:], ssum[:])
        w_sb = rows.tile([1, T], F32, tag="wsb")
        nc.vector.tensor_scalar_mul(w_sb[:], e_sb[:], rinv[:])
        nc.sync.dma_start(out=attn_out[b, :], in_=w_sb[:])

        sco = rows.tile([1, T], F32, tag="sco")
        nc.vector.tensor_scalar_add(sco[:], score_sb[:], vb_sb[:])
        nc.sync.dma_start(out=score_out[b, :], in_=sco[:])

        # context: ctx[h=c*128+p] = sum_t w[t] * vT[p, c, t]
        w16 = rows.tile([1, T], BF16, tag="w16")
        nc.vector.tensor_copy(w16[:], w_sb[:])
        w_bc = work.tile([128, T], BF16, tag="wbc")
        nc.gpsimd.partition_broadcast(w_bc[:], w16[:])
        ctx_sb = work.tile([128, HC], F32, tag="ctx")
        for c in range(HC):
            tts = work.tile([128, T], BF16, tag="tts", bufs=2)
            nc.vector.tensor_mul(tts[:], vT[:, c, :], w_bc[:])
            nc.vector.reduce_sum(ctx_sb[:, c:c + 1], tts[:], axis=AX)
        for c in range(HC):
            nc.sync.dma_start(out=ctx_out[b, c * 128:(c + 1) * 128],
                              in_=ctx_sb[:, c:c + 1])


def build_program(bl=BL, reps=1):
    nc = bacc.Bacc("TRN2", target_bir_lowering=False, debug=False,
                   enable_asserts=False)
    io = {}

    def inp(name, shape):
        io[name] = nc.dram_tensor(name, list(shape), F32, kind="ExternalInput").ap()

    def outp(name, shape):
        io[name] = nc.dram_tensor(name, list(shape), F32, kind="ExternalOutput").ap()

    inp("values", (bl, T, H))
    inp("query", (bl, H))
    inp("prev", (bl, T))
    inp("W1_w", (U, H))
    inp("W2_w", (U, H))
    inp("W1_b", (U,))
    inp("W2_b", (U,))
    inp("V_w", (1, U))
    inp("V_b", (1,))
    inp("conv_w", (KN, KW))
    inp("loc_proj", (U, KN))
    outp("ctx_out", (bl, H))
    outp("attn_out", (bl, T))
    outp("score_out", (bl, T))

    with tile.TileContext(nc) as tc, ExitStack() as ctx:
        build_kernel(ctx, nc, tc, io, bl=bl, reps=reps)
    nc.compile()
    return nc


def make_in_maps(query, values, prev_att, W1_w, W1_b, W2_w, W2_b, V_w, V_b,
                 conv_w, loc_proj_w, ncores=NCORES, bl=BL):
    f = lambda x: np.ascontiguousarray(np.asarray(x), dtype=np.float32)
    shared = {
        "W1_w": f(W1_w), "W2_w": f(W2_w), "W1_b": f(W1_b), "W2_b": f(W2_b),
        "V_w": f(V_w).reshape(1, U), "V_b": f(V_b).reshape(1),
        "conv_w": f(conv_w).reshape(KN, KW), "loc_proj": f(loc_proj_w),
    }
    in_maps = []
    for core in range(ncores):
        sl = slice(core * bl, (core + 1) * bl)
        m = dict(shared)
        m["values"] = f(values[sl])
        m["query"] = f(query[0, sl])
        m["prev"] = f(prev_att[sl, 0, :])
        in_maps.append(m)
    return in_maps


_PROGRAM = None


def _get_program():
    global _PROGRAM
    if _PROGRAM is None:
        _PROGRAM = build_program()
    return _PROGRAM


def kernel(query, values, prev_att, W1_w, W1_b, W2_w, W2_b, V_w, V_b,
           conv_w, loc_proj_w):
    nc = _get_program()
    in_maps = make_in_maps(query, values, prev_att, W1_w, W1_b, W2_w, W2_b,
                           V_w, V_b, conv_w, loc_proj_w)
    res = run_bass_kernel_spmd(nc, in_maps, list(range(NCORES)))
    ctx = np.concatenate([res.results[i]["ctx_out"] for i in range(NCORES)], 0)
    attn = np.concatenate([res.results[i]["attn_out"] for i in range(NCORES)], 0)
    score = np.concatenate([res.results[i]["score_out"] for i in range(NCORES)], 0)
    return (ctx.astype(np.float32), attn[:, :, None].astype(np.float32),
            score[:, :, None].astype(np.float32))
